# revision 12
# baseline (speedup 1.0000x reference)
"""Energy Transformer descent kernel for 8 Trainium2 NeuronCores.

Problem: 12 steps of gradient descent on
  E(x) = -(1/beta) sum logsumexp(beta q k^T) - 0.5 sum relu(g xi^T)^2,
  g = LayerNorm(x; gamma, delta), q = g Wq_h, k = g Wk_h.

Sharding: data-parallel over batch B=4 -> core pairs (2b, 2b+1); within a
pair, core j owns TOKENS j*256..(j+1)*256 (all 12 heads, all 3072 Hopfield
memories).  Attention queries / Hopfield rows / LayerNorm-backward are
computed for own tokens only; keys need all tokens, so the partial
dk^T (summed over own queries) is pair-ReduceScattered -- overlapped with
the Hopfield phase on the PE -- and the per-token dx halves are
pair-AllGathered at the end of the step.

The SPMD program is identical on both cores of a pair: token ownership
enters only through (a) a per-core one-hot selection matrix input `sel`
([N, 256], data not code) used to project own-token rows via matmuls, and
(b) the rank-major layout of the ReduceScatter/AllGather buffers.

Host-side preprocessing folds gamma and the attention scale into the
weights (delta must be zero, which the problem guarantees):
  wq = sqrt(beta) diag(gamma) Wq   (forward projections, likewise wk)
  wqt = (1/sqrt(beta)) (diag(gamma) Wq)^T   (gradient projections)
  xi' = xi diag(gamma)
All matmuls run in bf16 (fp32 PSUM accumulation); fp8 was measured to
break the 2e-2 gate.  Softmax normalisation is folded into the P-transpose
by multiplying with diag(1/rowsum) instead of the identity, and into the
dk^T matmul by pre-scaling q rows.
"""

import numpy as np

import concourse.bass as bass
import concourse.tile as tile
from concourse import bacc, mybir

STEPS = 12
ALPHA = 0.125
EPS = 1e-5
B, N, D, H, HD, M = 4, 512, 768, 12, 64, 3072
P = 128
NT = N // P  # 4 full-token chunks
OC = 2       # own-token chunks (256 own tokens)
NL = OC * P
DT = D // P  # 6 embed chunks
EW = H * HD  # 768 head width (all heads)
ET = EW // P  # 6 head-dim chunks
MT = M // P  # 24 memory chunks
F32 = mybir.dt.float32
BF16 = mybir.dt.bfloat16
AF = mybir.ActivationFunctionType
OP = mybir.AluOpType

REPLICA_GROUPS = [[0, 1], [2, 3], [4, 5], [6, 7]]


def build_kernel(steps=STEPS, with_cc=True, debug_dump=False):
    nc = bacc.Bacc("TRN2", target_bir_lowering=False, debug=False, num_devices=8)

    x_in = nc.declare_dram_parameter("x", [N, D], F32, isOutput=False)
    sel_d = nc.declare_dram_parameter("sel", [N, NL], BF16, isOutput=False)
    wq_d = nc.declare_dram_parameter("wq", [D, EW], BF16, isOutput=False)
    wk_d = nc.declare_dram_parameter("wk", [D, EW], BF16, isOutput=False)
    wqt_d = nc.declare_dram_parameter("wqt", [EW, D], BF16, isOutput=False)
    wkt_d = nc.declare_dram_parameter("wkt", [EW, D], BF16, isOutput=False)
    xi_d = nc.declare_dram_parameter("xi", [M, D], BF16, isOutput=False)
    xit_d = nc.declare_dram_parameter("xit", [D, M], BF16, isOutput=False)
    x_out = nc.declare_dram_parameter("x_out", [N, D], F32, isOutput=True)
    dbg = {}
    if debug_dump:
        for nm, shp, dt_ in (("xhat", [N, D], BF16), ("xh_own", [NL, D], F32),
                             ("gT_own", [D, NL], BF16), ("q_own", [NL, EW], BF16),
                             ("kT", [EW, N], BF16), ("U0", [NL, N], BF16),
                             ("PT0", [N, NL], BF16), ("dqTst", [EW, NL], BF16),
                             ("dkTst", [EW, N], BF16), ("dkT_own", [EW, NL], BF16),
                             ("dgTs", [D, NL], BF16), ("dg_own", [NL, D], F32),
                             ("rstd_own", [NL, 1], F32), ("s01", [NL, 2], F32),
                             ("dxb", [NL, D], BF16)):
            dbg[nm] = nc.declare_dram_parameter("o_" + nm, shp, dt_, isOutput=True)

    def dump(nm, ap, pdim):
        if not debug_dump:
            return
        nc.sync.dma_start(out=dbg[nm].rearrange("(a p) b -> p a b", p=pdim), in_=ap)

    with tile.TileContext(nc) as tc:
        import contextlib

        with contextlib.ExitStack() as ctx:
            consts = ctx.enter_context(tc.tile_pool(name="consts", bufs=1))
            work = ctx.enter_context(tc.tile_pool(name="work", bufs=1))
            upool = ctx.enter_context(tc.tile_pool(name="upool", bufs=2))
            ptool = ctx.enter_context(tc.tile_pool(name="ptool", bufs=2))
            rtp = ctx.enter_context(tc.tile_pool(name="rtp", bufs=3))
            stats = ctx.enter_context(tc.tile_pool(name="stats", bufs=4))
            # PSUM: psdg 3 banks + pw 2 + ps2 3 = 8
            psdg = ctx.enter_context(tc.tile_pool(name="psdg", bufs=1, space="PSUM"))
            pw = ctx.enter_context(tc.tile_pool(name="pw", bufs=2, space="PSUM"))
            ps2 = ctx.enter_context(tc.tile_pool(name="ps2", bufs=3, space="PSUM"))
            drp = ctx.enter_context(tc.tile_pool(name="drp", bufs=2, space="DRAM"))

            # ---- resident tensors ----
            wq_sb = consts.tile([P, DT, EW], BF16)
            nc.sync.dma_start(out=wq_sb[:], in_=wq_d.rearrange("(dt p) e -> p dt e", p=P))
            wk_sb = consts.tile([P, DT, EW], BF16)
            nc.sync.dma_start(out=wk_sb[:], in_=wk_d.rearrange("(dt p) e -> p dt e", p=P))
            wqt_sb = consts.tile([P, ET, D], BF16)
            nc.sync.dma_start(out=wqt_sb[:], in_=wqt_d.rearrange("(et p) d -> p et d", p=P))
            wkt_sb = consts.tile([P, ET, D], BF16)
            nc.sync.dma_start(out=wkt_sb[:], in_=wkt_d.rearrange("(et p) d -> p et d", p=P))
            xi_sb = consts.tile([P, MT, D], BF16)
            nc.sync.dma_start(out=xi_sb[:], in_=xi_d.rearrange("(mt p) d -> p mt d", p=P))
            xit_sb = consts.tile([P, DT, M], BF16)
            nc.sync.dma_start(out=xit_sb[:], in_=xit_d.rearrange("(dt p) m -> p dt m", p=P))
            sel_sb = consts.tile([P, NT, NL], BF16)
            nc.sync.dma_start(out=sel_sb[:], in_=sel_d.rearrange("(nt p) c -> p nt c", p=P))
            sel32 = consts.tile([P, NT, NL], F32)
            nc.vector.tensor_copy(out=sel32[:], in_=sel_sb[:])
            x_sb = consts.tile([P, NT, D], F32)
            nc.sync.dma_start(out=x_sb[:], in_=x_in.rearrange("(nt p) d -> p nt d", p=P))

            from concourse.masks import make_identity

            ident_f = consts.tile([P, P], F32)
            make_identity(nc, ident_f[:])
            ident_b = consts.tile([P, P], BF16)
            nc.vector.tensor_copy(out=ident_b[:], in_=ident_f[:])
            eps_t = consts.tile([P, 1], F32)
            nc.vector.memset(eps_t[:], EPS)
            zl_t = consts.tile([1, P], BF16)
            nc.vector.memset(zl_t[:], 0.0)
            zr_t = consts.tile([1, N], BF16)
            nc.vector.memset(zr_t[:], 0.0)

            for step in range(steps):
                # ======== LayerNorm forward (full tokens) ========
                xhat = work.tile([P, NT, D], BF16, tag="xhat")
                rr_t = stats.tile([P, NT], F32, tag="rr")
                for nt in range(NT):
                    xt = x_sb[:, nt, :]
                    st = stats.tile([P, 3, 6], F32, tag="bnst")
                    xg = xt.rearrange("p (g s) -> p g s", s=256)
                    for gs in range(3):
                        nc.vector.bn_stats(out=st[:, gs, :], in_=xg[:, gs, :])
                    mv = stats.tile([P, 2], F32, tag="mv")
                    nc.vector.bn_aggr(out=mv[:], in_=st[:])
                    rrx = rr_t[:, nt : nt + 1]
                    nc.scalar.activation(out=rrx, in_=mv[:, 1:2], func=AF.Sqrt, bias=eps_t[:], scale=1.0)
                    nc.vector.reciprocal(out=rrx, in_=rrx)
                    nmu = stats.tile([P, 1], F32, tag="nmu")
                    nc.vector.scalar_tensor_tensor(
                        out=nmu[:], in0=mv[:, 0:1], scalar=-1.0, in1=rrx, op0=OP.mult, op1=OP.mult,
                    )
                    nc.scalar.activation(
                        out=xhat[:, nt, :], in_=xt, func=AF.Identity, scale=rrx, bias=nmu[:],
                    )

                # ======== gT = xhat^T (full) ========
                gT = work.tile([P, DT, N], BF16, tag="gT")
                for dt in range(DT):
                    pg = ps2.tile([P, N], BF16, tag="ps2")
                    for nt in range(NT):
                        nc.tensor.transpose(pg[:, nt * P : (nt + 1) * P], xhat[:, nt, dt * P : (dt + 1) * P], ident_b[:])
                    nc.vector.tensor_copy(out=gT[:, dt, :], in_=pg[:])

                if debug_dump and step == 0:
                    dump("xhat", xhat[:], P)
                # ======== own-token selection (via sel matmuls) ========
                # xhat_own[c, d] = sum_n sel[n, c] * xhat[n, d]
                xhat_own = work.tile([P, OC, D], F32, tag="xh_own")
                xhat_own_b = work.tile([P, OC, D], BF16, tag="ocd_b")
                for oc in range(OC):
                    pa = pw.tile([P, 512], F32, tag="pw")
                    pb = ps2.tile([P, 512], F32, tag="ps2")
                    for nt in range(NT):
                        lh = sel_sb[:, nt, oc * P : (oc + 1) * P]
                        nc.tensor.matmul(pa[:], lh, xhat[:, nt, 0:512], start=(nt == 0), stop=(nt == NT - 1))
                        nc.tensor.matmul(pb[:, :256], lh, xhat[:, nt, 512:768], start=(nt == 0), stop=(nt == NT - 1))
                    nc.scalar.activation(out=xhat_own[:, oc, 0:512], in_=pa[:], func=AF.Copy)
                    nc.scalar.activation(out=xhat_own[:, oc, 512:768], in_=pb[:, :256], func=AF.Copy)
                    nc.vector.tensor_copy(out=xhat_own_b[:, oc, 0:512], in_=pa[:])
                    nc.vector.tensor_copy(out=xhat_own_b[:, oc, 512:768], in_=pb[:, :256])
                # rstd_own (exact, fp32 matmul on [.,1])
                rstd_own = stats.tile([P, OC], F32, tag="rstd_own")
                for oc in range(OC):
                    pr = ps2.tile([P, 1], F32, tag="ps2")
                    for nt in range(NT):
                        nc.tensor.matmul(
                            pr[:], sel32[:, nt, oc * P : (oc + 1) * P], rr_t[:, nt : nt + 1],
                            start=(nt == 0), stop=(nt == NT - 1),
                        )
                    nc.vector.tensor_copy(out=rstd_own[:, oc : oc + 1], in_=pr[:])
                # gT_own = transpose(xhat_own)
                gT_own = work.tile([P, DT, NL], BF16, tag="gT_own")
                for dt in range(DT):
                    pg = ps2.tile([P, NL], BF16, tag="ps2")
                    for oc in range(OC):
                        nc.tensor.transpose(pg[:, oc * P : (oc + 1) * P], xhat_own_b[:, oc, dt * P : (dt + 1) * P], ident_b[:])
                    nc.vector.tensor_copy(out=gT_own[:, dt, :], in_=pg[:])

                if debug_dump and step == 0:
                    dump("xh_own", xhat_own[:], P)
                    dump("gT_own", gT_own[:], P)
                    nc.sync.dma_start(
                        out=dbg["rstd_own"].rearrange("(c p) o -> p c o", p=P),
                        in_=rstd_own[:].rearrange("p (c o) -> p c o", o=1),
                    )
                # ======== projections ========
                # q (own tokens): q[c, e] = sum_d xhat_own[c, d] wq[d, e]
                q_own = work.tile([P, OC, EW], BF16, tag="q_own")
                for oc in range(OC):
                    pa = pw.tile([P, 512], F32, tag="pw")
                    pb = ps2.tile([P, 512], F32, tag="ps2")
                    for dt in range(DT):
                        lh = gT_own[:, dt, oc * P : (oc + 1) * P]
                        nc.tensor.matmul(pa[:, :384], lh, wq_sb[:, dt, 0:384], start=(dt == 0), stop=(dt == DT - 1))
                        nc.tensor.matmul(pb[:, :384], lh, wq_sb[:, dt, 384:768], start=(dt == 0), stop=(dt == DT - 1))
                    nc.vector.tensor_copy(out=q_own[:, oc, 0:384], in_=pa[:, :384])
                    nc.vector.tensor_copy(out=q_own[:, oc, 384:768], in_=pb[:, :384])
                # k (all tokens)
                k_sb = work.tile([P, NT, EW], BF16, tag="k")
                for nt in range(NT):
                    pa = pw.tile([P, 512], F32, tag="pw")
                    pb = ps2.tile([P, 512], F32, tag="ps2")
                    for dt in range(DT):
                        lh = gT[:, dt, nt * P : (nt + 1) * P]
                        nc.tensor.matmul(pa[:, :384], lh, wk_sb[:, dt, 0:384], start=(dt == 0), stop=(dt == DT - 1))
                        nc.tensor.matmul(pb[:, :384], lh, wk_sb[:, dt, 384:768], start=(dt == 0), stop=(dt == DT - 1))
                    nc.vector.tensor_copy(out=k_sb[:, nt, 0:384], in_=pa[:, :384])
                    nc.vector.tensor_copy(out=k_sb[:, nt, 384:768], in_=pb[:, :384])
                # qT (own), kT (full)
                qT = work.tile([P, ET, NL], BF16, tag="qT")
                for et in range(ET):
                    pg = ps2.tile([P, NL], BF16, tag="ps2")
                    for oc in range(OC):
                        nc.tensor.transpose(pg[:, oc * P : (oc + 1) * P], q_own[:, oc, et * P : (et + 1) * P], ident_b[:])
                    nc.vector.tensor_copy(out=qT[:, et, :], in_=pg[:])
                kT = work.tile([P, ET, N], BF16, tag="kT")
                for et in range(ET):
                    pg = ps2.tile([P, N], BF16, tag="ps2")
                    for nt in range(NT):
                        nc.tensor.transpose(pg[:, nt * P : (nt + 1) * P], k_sb[:, nt, et * P : (et + 1) * P], ident_b[:])
                    nc.vector.tensor_copy(out=kT[:, et, :], in_=pg[:])

                if debug_dump and step == 0:
                    dump("q_own", q_own[:], P)
                    dump("kT", kT[:], P)
                # ======== attention heads ========
                dqTst = work.tile([P, ET, NL], BF16, tag="dqTst")
                dkTst = work.tile([P, ET, N], BF16, tag="dkTst")
                pq = pk = None
                for h in range(H):
                    et, eo = h // 2, (h % 2) * HD
                    Un = upool.tile([P, OC, N], BF16, tag="Un")
                    sm = stats.tile([P, OC], F32, tag="sm")
                    for oc in range(OC):
                        sc = ps2.tile([P, 512], F32, tag="ps2")
                        nc.tensor.matmul(
                            sc[:], qT[eo : eo + HD, et, oc * P : (oc + 1) * P],
                            kT[eo : eo + HD, et, :], start=True, stop=True,
                        )
                        nc.scalar.activation(
                            out=Un[:, oc, :], in_=sc[:], func=AF.Exp, bias=0.0, scale=1.0,
                            accum_out=sm[:, oc : oc + 1],
                        )
                    nc.vector.reciprocal(out=sm[:], in_=sm[:])
                    # diag(1/rowsum) per own chunk; PT = U^T @ diag (normalised)
                    dg_m = stats.tile([P, OC, P], BF16, tag="diag")
                    for oc in range(OC):
                        nc.vector.tensor_scalar_mul(out=dg_m[:, oc, :], in0=ident_b[:], scalar1=sm[:, oc : oc + 1])
                        nc.vector.tensor_scalar_mul(
                            out=q_own[:, oc, h * HD : (h + 1) * HD],
                            in0=q_own[:, oc, h * HD : (h + 1) * HD], scalar1=sm[:, oc : oc + 1],
                        )
                    if debug_dump and step == 0 and h == 0:
                        dump("U0", Un[:], P)
                    PTn = ptool.tile([P, NT, NL], BF16, tag="PTn")
                    for mt in range(NT):
                        pp = ps2.tile([P, NL], F32, tag="ps2")
                        for oc in range(OC):
                            nc.tensor.matmul(
                                pp[:, oc * P : (oc + 1) * P], Un[:, oc, mt * P : (mt + 1) * P],
                                dg_m[:, oc, :], start=True, stop=True,
                            )
                        nc.scalar.activation(out=PTn[:, mt, :], in_=pp[:], func=AF.Copy)
                    if debug_dump and step == 0 and h == 0:
                        dump("PT0", PTn[:], P)
                    # dqT_h[e, c] = sum_m k[m, e] PT[m, c]   (psum-packed per head pair)
                    if h % 2 == 0:
                        pq = pw.tile([P, NL], F32, tag="pw")
                        pk = pw.tile([P, N], F32, tag="pw")
                    for mt in range(NT):
                        nc.tensor.matmul(
                            pq[eo : eo + HD, :], k_sb[:, mt, h * HD : (h + 1) * HD], PTn[:, mt, :],
                            start=(mt == 0), stop=(mt == NT - 1),
                        )
                    # dkT_h[e, n] = sum_c q'[c, e] U[c, n]
                    for oc in range(OC):
                        nc.tensor.matmul(
                            pk[eo : eo + HD, :], q_own[:, oc, h * HD : (h + 1) * HD], Un[:, oc, :],
                            start=(oc == 0), stop=(oc == OC - 1),
                        )
                    if h % 2 == 1:
                        nc.vector.tensor_copy(out=dqTst[:, et, :], in_=pq[:])
                        nc.vector.tensor_copy(out=dkTst[:, et, :], in_=pk[:])

                if debug_dump and step == 0:
                    dump("dqTst", dqTst[:], P)
                    dump("dkTst", dkTst[:], P)
                # ======== pair ReduceScatter of dk^T (overlaps Hopfield) ========
                if with_cc:
                    rs_in = drp.tile([2, EW, NL], BF16, tag="rs_in")
                    rs_out = drp.tile([EW, NL], BF16, tag="rs_out")
                    for r in range(2):
                        nc.sync.dma_start(
                            out=rs_in[r].rearrange("(et p) n -> p et n", p=P),
                            in_=dkTst[:, :, r * NL : (r + 1) * NL],
                        )
                    nc.gpsimd.collective_compute(
                        "ReduceScatter", OP.add, replica_groups=REPLICA_GROUPS,
                        ins=[rs_in.opt()], outs=[rs_out.opt()],
                    )
                    dkT_own = work.tile([P, ET, NL], BF16, tag="dkT_own")
                    nc.sync.dma_start(out=dkT_own[:], in_=rs_out.rearrange("(et p) n -> p et n", p=P))
                else:
                    dkT_own = work.tile([P, ET, NL], BF16, tag="dkT_own")
                    nc.vector.tensor_copy(out=dkT_own[:], in_=dkTst[:, :, 0:NL])

                if debug_dump and step == 0:
                    dump("dkT_own", dkT_own[:], P)
                # ======== Hopfield (own tokens, all memories) ========
                # dgT accumulation: 3 psum banks, halves = d-chunk pairs.
                dgTb = [psdg.tile([P, N], F32, tag=f"dgT{b}", name=f"dgT{b}") for b in range(3)]
                for b in range(3):
                    nc.tensor.matmul(dgTb[b][:], zl_t[:], zr_t[:], start=True, stop=False)
                for mt in range(MT):
                    hp = ps2.tile([P, NL], F32, tag="ps2")
                    for dt in range(DT):
                        nc.tensor.matmul(
                            hp[:], xit_sb[:, dt, mt * P : (mt + 1) * P], gT_own[:, dt, :],
                            start=(dt == 0), stop=(dt == DT - 1),
                        )
                    RT = rtp.tile([P, NL], BF16, tag="RT")
                    nc.scalar.activation(out=RT[:], in_=hp[:], func=AF.Relu)
                    for dt in range(DT):
                        b, half = dt // 2, dt % 2
                        nc.tensor.matmul(
                            dgTb[b][:, half * NL : (half + 1) * NL],
                            xi_sb[:, mt, dt * P : (dt + 1) * P], RT[:],
                            start=False, stop=False,
                        )
                # dq-path: dgT[d, c] += wqt[e, d] dqT[e, c]
                for dt in range(DT):
                    b, half = dt // 2, dt % 2
                    for et in range(ET):
                        nc.tensor.matmul(
                            dgTb[b][:, half * NL : (half + 1) * NL],
                            wqt_sb[:, et, dt * P : (dt + 1) * P], dqTst[:, et, :],
                            start=False, stop=False,
                        )
                # dk-path (waits on ReduceScatter result)
                for dt in range(DT):
                    b, half = dt // 2, dt % 2
                    for et in range(ET):
                        nc.tensor.matmul(
                            dgTb[b][:, half * NL : (half + 1) * NL],
                            wkt_sb[:, et, dt * P : (dt + 1) * P], dkT_own[:, et, :],
                            start=False, stop=(et == ET - 1 and half == 1),
                        )

                # ======== dg -> [own-n, d]; LayerNorm backward; dx ========
                dgTs = work.tile([P, DT, NL], BF16, tag="qT")
                for b in range(3):
                    nc.vector.tensor_copy(
                        out=dgTs[:, 2 * b : 2 * b + 2, :].rearrange("p t n -> p (t n)"),
                        in_=dgTb[b][:],
                    )
                if debug_dump and step == 0:
                    dump("dgTs", dgTs[:], P)
                dg_own = work.tile([P, OC, D], F32, tag="dg_own")
                dxb = work.tile([P, OC, D], BF16, tag="ocd_b")
                m1s = stats.tile([P, OC], F32, tag="m1s")
                u2s = stats.tile([P, OC], F32, tag="u2s")
                for oc in range(OC):
                    pg = ps2.tile([P, D], BF16, tag="ps2")
                    for dt in range(DT):
                        nc.tensor.transpose(pg[:, dt * P : (dt + 1) * P], dgTs[:, dt, oc * P : (oc + 1) * P], ident_b[:])
                    # dy' = rstd * dg  (+ row-sum accumulation for <dy>)
                    nc.vector.scalar_tensor_tensor(
                        out=dg_own[:, oc, :], in0=pg[:], scalar=rstd_own[:, oc : oc + 1],
                        in1=xhat_own[:, oc, :], op0=OP.mult, op1=OP.bypass,
                        accum_out=m1s[:, oc : oc + 1],
                    )
                    prod = work.tile([P, D], F32, tag="prod")
                    nc.vector.scalar_tensor_tensor(
                        out=prod[:], in0=dg_own[:, oc, :], scalar=1.0, in1=xhat_own[:, oc, :],
                        op0=OP.mult, op1=OP.mult, accum_out=u2s[:, oc : oc + 1],
                    )
                s01 = stats.tile([P, OC, 2], F32, tag="s01")
                nc.vector.tensor_scalar(
                    out=s01[:, :, 0], in0=u2s[:], scalar1=1.0 / D, scalar2=None, op0=OP.mult,
                )
                nc.vector.tensor_scalar(
                    out=s01[:, :, 1], in0=m1s[:], scalar1=1.0 / D, scalar2=None, op0=OP.mult,
                )
                for oc in range(OC):
                    nc.vector.ln_bwd_dx(
                        out=dxb[:, oc, :], dy=dg_own[:, oc, :], x_hat=xhat_own[:, oc, :],
                        mean_dyx=s01[:, oc, 0:1], mean_dy=s01[:, oc, 1:2], scale=ALPHA,
                    )

                if debug_dump and step == 0:
                    dump("dg_own", dg_own[:], P)
                    dump("s01", s01[:], P)
                    dump("dxb", dxb[:], P)
                # ======== pair AllGather of dx; update x ========
                dxg = work.tile([P, NT, D], BF16, tag="k")
                if with_cc:
                    ag_in = drp.tile([NL, D], BF16, tag="ag_in")
                    ag_out = drp.tile([N, D], BF16, tag="ag_out")
                    nc.sync.dma_start(out=ag_in.rearrange("(oc p) d -> p oc d", p=P), in_=dxb[:])
                    nc.gpsimd.collective_compute(
                        "AllGather", OP.bypass, replica_groups=REPLICA_GROUPS,
                        ins=[ag_in.opt()], outs=[ag_out.opt()],
                    )
                    nc.sync.dma_start(out=dxg[:], in_=ag_out.rearrange("(nt p) d -> p nt d", p=P))
                else:
                    nc.vector.memset(dxg[:], 0.0)
                    nc.vector.tensor_copy(out=dxg[:, 0:OC, :].rearrange("p t d -> p (t d)"), in_=dxb[:].rearrange("p t d -> p (t d)"))
                for nt in range(NT):
                    nc.vector.scalar_tensor_tensor(
                        out=x_sb[:, nt, :], in0=dxg[:, nt, :], scalar=1.0, in1=x_sb[:, nt, :],
                        op0=OP.mult, op1=OP.add,
                    )

            for nt in range(NT):
                nc.sync.dma_start(out=x_out[nt * P : (nt + 1) * P, :], in_=x_sb[:, nt, :])

    nc.compile()
    return nc


def _prep_inputs(x, gamma, delta, Wq, Wk, xi):
    """Build the 8 per-core input dicts (host-side sharding + weight folding)."""
    assert np.allclose(delta, 0.0), "kernel requires delta == 0"
    import ml_dtypes

    bf = ml_dtypes.bfloat16
    beta_sqrt = np.float32(1.0 / np.sqrt(np.sqrt(np.float32(HD))))
    g = gamma.astype(np.float32)
    wq = ((Wq * g[None, :, None]).transpose(1, 0, 2).reshape(D, EW) * beta_sqrt).astype(bf)
    wk = ((Wk * g[None, :, None]).transpose(1, 0, 2).reshape(D, EW) * beta_sqrt).astype(bf)
    wqt = ((Wq * g[None, :, None]).transpose(0, 2, 1).reshape(EW, D) / beta_sqrt).astype(bf)
    wkt = ((Wk * g[None, :, None]).transpose(0, 2, 1).reshape(EW, D) / beta_sqrt).astype(bf)
    xi_f = (xi * g[None, :]).astype(np.float32)
    xi_b = np.ascontiguousarray(xi_f).astype(bf)
    xit_b = np.ascontiguousarray(xi_f.T).astype(bf)
    sels = []
    for j in range(2):
        s = np.zeros((N, NL), dtype=bf)
        s[np.arange(j * NL, (j + 1) * NL), np.arange(NL)] = 1
        sels.append(s)
    in_maps = []
    for c in range(8):
        b, j = c // 2, c % 2
        in_maps.append(
            {
                "x": np.ascontiguousarray(x[b]),
                "sel": sels[j],
                "wq": wq, "wk": wk, "wqt": wqt, "wkt": wkt,
                "xi": xi_b, "xit": xit_b,
            }
        )
    return in_maps


_NC_CACHE = {}


def _get_nc(steps=STEPS, with_cc=True):
    key = (steps, with_cc)
    if key not in _NC_CACHE:
        _NC_CACHE[key] = build_kernel(steps, with_cc)
    return _NC_CACHE[key]


def kernel(x, gamma, delta, Wq, Wk, xi):
    from concourse.bass_utils import run_bass_kernel_spmd

    x = np.asarray(x, dtype=np.float32)
    in_maps = _prep_inputs(
        x,
        np.asarray(gamma, np.float32),
        np.asarray(delta, np.float32),
        np.asarray(Wq, np.float32),
        np.asarray(Wk, np.float32),
        np.asarray(xi, np.float32),
    )
    nc = _get_nc()
    res = run_bass_kernel_spmd(nc, in_maps, list(range(8)))
    out = np.stack([res.results[2 * b]["x_out"] for b in range(B)], axis=0)
    return out.astype(np.float32)
